# revision 1
# baseline (speedup 1.0000x reference)
"""Trainium2 Bass kernel for nn_EquivariantProductBasisBlock.

Math: per (node n, channel c), out[d] is a degree-3 polynomial in the
9-vector x[n,c,:] with coefficients depending on (element_type(n), c),
followed by a per-l channel-mixing linear layer and the sc skip-add.

Device mapping (per core, 256 nodes, data-parallel over 8 cores):
  1. monomials (219 per (n,c)) on DVE/GPSIMD in node-partition layout
  2. DMA-xbar transpose to monomial-partition layout
  3. PE matmul with the symmetrized coefficient matrix Usym [219 x 63]
  4. DMA-xbar transpose back; multiply by per-(n,c) element weights (wy)
  5. PE transpose + lin_w matmul contracting channels; add sc; DMA out.
"""
import numpy as np
from itertools import permutations

import bass_rust
import concourse.bass as bass
import concourse.bacc as bacc
import concourse.tile as tile
import concourse.mybir as mybir
from concourse.bass_utils import run_bass_kernel_spmd
from concourse.vector_clock import ScopedClock

# ---------------- problem constants (hardcoded per contest rules) ----------
N, C, E = 2048, 128, 10
NCORES = 8
NSH = N // NCORES            # 256 nodes per core
DIMS = [1, 3, 5]
P1, P2, P3 = 1, 2, 4
BL = [0, 7, 28]              # 7-packed col base per l  (63 cols)
BL8 = [0, 8, 32]             # 8-packed col base per l  (72 cols)
FP32 = mybir.dt.float32
BF16 = mybir.dt.bfloat16

# monomial layout inside the mono tile's per-c 224-run:
#   [0:165 deg3 | 165:168 zero-pad | 168:213 deg2 | 213:222 deg1(x) | 222:256 zero-pad]
MPAD = 256
D2OFF = 168
D1OFF = 213
# global usym row layout (219): [deg3 0:165 | deg2 165:210 | deg1 210:219]


def tri(v):
    return v * (v + 1) // 2


_B3 = [0]
for v in range(9):
    _B3.append(_B3[-1] + (v + 1) * (9 - v))
assert _B3[9] == 165


def m3_idx(u, v, w):
    return _B3[v] + u * (9 - v) + (w - v)


# ------------------------- host precompute --------------------------------

def build_usym(inputs):
    """usym [219, 64] float64; col j = BL[l] + d*7 + q (q: 0-3 nu3, 4-5 nu2, 6 nu1)."""
    usym = np.zeros((219, 64), dtype=np.float64)
    for li in range(3):
        U1 = np.asarray(inputs[f"U1_{li}"], dtype=np.float64)
        U2 = np.asarray(inputs[f"U2_{li}"], dtype=np.float64)
        U3 = np.asarray(inputs[f"U3_{li}"], dtype=np.float64)
        for d in range(DIMS[li]):
            base = BL[li] + d * 7
            for p in range(P3):
                T = U3[d, :, :, :, p]
                for u in range(9):
                    for v in range(u, 9):
                        for w in range(v, 9):
                            s = sum(T[pm] for pm in set(permutations((u, v, w))))
                            usym[m3_idx(u, v, w), base + p] = s
            for p in range(P2):
                T = U2[d, :, :, p]
                for u in range(9):
                    for v in range(u, 9):
                        s = sum(T[pm] for pm in set(permutations((u, v))))
                        usym[165 + tri(v) + u, base + 4 + p] = s
            for u in range(9):
                usym[210 + u, base + 6] = U1[d, u, 0]
    return usym


def build_usym_tiles(usym):
    """usymA [128,64] rows=m 0:128; usymB [128,64] rows 0:94 laid out to match
    the chunk-1 transpose: [0:37 m128:165 | 37:40 zero | 40:85 deg2 | 85:94 deg1]."""
    a = np.zeros((128, 64), dtype=np.float32)
    b = np.zeros((128, 64), dtype=np.float32)
    a[:, :] = usym[0:128]
    b[0:37] = usym[128:165]
    b[40:85] = usym[165:210]
    b[85:94] = usym[210:219]
    return a, b


def build_wall(inputs):
    """wall [16, 3072] f32: rows 0:10 = element e; col = l*1024 + q*128 + c, q8 (q=7 zero)."""
    wall = np.zeros((16, 3072), dtype=np.float32)
    for li in range(3):
        W1 = np.asarray(inputs["W1"][li])  # [E,P1,C]
        W2 = np.asarray(inputs["W2"][li])
        W3 = np.asarray(inputs["W3"][li])
        for q in range(7):
            if q < 4:
                w = W3[:, q, :]
            elif q < 6:
                w = W2[:, q - 4, :]
            else:
                w = W1[:, 0, :]
            wall[0:E, li * 1024 + q * 128:li * 1024 + q * 128 + C] = w
    return wall


def build_lw2(inputs):
    """lw2 [128, 768] f32: six stationaries s=(l,half): col s*128+f;
    rows r: c = half*64 + (r % 64); value lin_w[l][c,f]/sqrt(C)."""
    lw2 = np.zeros((128, 768), dtype=np.float32)
    isq = 1.0 / np.sqrt(np.float32(C))
    lw = np.asarray(inputs["lin_w"])  # [3, C, C]
    for li in range(3):
        for half in range(2):
            s = li * 2 + half
            blk = lw[li, half * 64:(half + 1) * 64, :] * isq  # [64, 128]
            lw2[0:64, s * 128:(s + 1) * 128] = blk
            lw2[64:128, s * 128:(s + 1) * 128] = blk
    return lw2


# --------------------------- device program --------------------------------

class _TC(tile.TileContext):
    """TileContext with the final sync-engine drain split into 1-wait drains
    (this walrus build rejects >1 sem wait on a sync CTRL instruction)."""

    def _drain_and_barrier(self, tick_clock, wait_clock):
        drain_inst = self.nc.sync.drain()
        wait_clock.add_sem_waits(
            drain_inst.ins, ScopedClock({None: tick_clock.global_clock})
        )
        si = drain_inst.ins.sync_info
        waits = list(si.on_wait or []) if si else []
        if len(waits) > 1:
            si.on_wait = waits[:1]
            for w in waits[1:]:
                extra = self.nc.sync.drain()
                extra.ins.sync_info = bass_rust.SyncInfo(on_wait=[w], on_update=[])
        self.nc.all_engine_barrier()
        assert self.sems is not None
        popped = self.nc._tile_sem_poison_stack.pop()
        assert popped is self._sem_poison
        self.nc.clear_and_free_semaphores(list(self.sems.allocated().values()))
        self.nc.all_engine_barrier()


def _raw(ap_like, extra_offset, dims):
    """Build a raw AP on the same tensor: dims = [[step,count],...] incl. partition dim."""
    base = ap_like[:, :] if not isinstance(ap_like, bass.AP) else ap_like
    return bass.AP(tensor=base.tensor, offset=base.offset + extra_offset, ap=dims)


def build_program():
    nc = bacc.Bacc("TRN2", target_bir_lowering=False, debug=False)

    xin = nc.dram_tensor("xin", [NSH, 1152], FP32, kind="ExternalInput").ap()
    scin = nc.dram_tensor("scin", [NSH, 1152], FP32, kind="ExternalInput").ap()
    ain = nc.dram_tensor("ain", [NSH, 16], FP32, kind="ExternalInput").ap()
    usymA_d = nc.dram_tensor("usymA", [128, 64], BF16, kind="ExternalInput").ap()
    usymB_d = nc.dram_tensor("usymB", [128, 64], BF16, kind="ExternalInput").ap()
    wall_d = nc.dram_tensor("wall", [16, 3072], BF16, kind="ExternalInput").ap()
    lw2_d = nc.dram_tensor("lw2", [128, 768], BF16, kind="ExternalInput").ap()
    id32_d = nc.dram_tensor("id32", [128, 128], FP32, kind="ExternalInput").ap()
    id16_d = nc.dram_tensor("id16", [128, 128], BF16, kind="ExternalInput").ap()
    yout = nc.dram_tensor("yout", [NSH, 1152], FP32, kind="ExternalOutput").ap()

    from contextlib import ExitStack
    with _TC(nc) as tc, ExitStack() as ctx:
        consts = ctx.enter_context(tc.tile_pool(name="consts", bufs=1))
        usymA = consts.tile([128, 64], BF16)
        usymB = consts.tile([128, 64], BF16)
        wall = consts.tile([16, 3072], BF16)
        lw2 = consts.tile([128, 768], BF16)
        id32 = consts.tile([128, 128], FP32)
        id16 = consts.tile([128, 128], BF16)
        for t, d in [(usymA, usymA_d), (usymB, usymB_d), (wall, wall_d),
                     (lw2, lw2_d), (id32, id32_d), (id16, id16_d)]:
            nc.sync.dma_start(out=t, in_=d)

        io = ctx.enter_context(tc.tile_pool(name="io", bufs=2))
        io1 = ctx.enter_context(tc.tile_pool(name="io1", bufs=1))
        mono_p = ctx.enter_context(tc.tile_pool(name="mono", bufs=2))
        mt_p = ctx.enter_context(tc.tile_pool(name="mt", bufs=1))
        g_p = ctx.enter_context(tc.tile_pool(name="g", bufs=1))
        small_p = ctx.enter_context(tc.tile_pool(name="small", bufs=1))
        ps_gps = ctx.enter_context(tc.tile_pool(name="psg", bufs=2, space="PSUM"))
        ps_pt = ctx.enter_context(tc.tile_pool(name="psp", bufs=1, space="PSUM"))
        ps_misc = ctx.enter_context(tc.tile_pool(name="psm", bufs=2, space="PSUM"))

        for chunk in range(2):
            n0 = chunk * 128
            # ---- loads
            x32 = io.tile([128, 1152], FP32, tag="x32")
            nc.sync.dma_start(out=x32, in_=xin[n0:n0 + 128, :])
            sc32 = io1.tile([128, 1152], FP32, tag="sc32")
            nc.sync.dma_start(out=sc32, in_=scin[n0:n0 + 128, :])
            a32 = io.tile([128, 16], FP32, tag="a32")
            nc.sync.dma_start(out=a32[:, 0:10], in_=ain[n0:n0 + 128, 0:10])

            # ---- wy = one_hot @ wall   -> wy16 [128n, 3072] (l,q8,c)
            aT_ps = ps_misc.tile([16, 128], FP32, tag="psmisc")
            nc.tensor.transpose(aT_ps[0:10, :], a32[:, 0:10], id32)
            aT16 = small_p.tile([16, 128], BF16, tag="aT16")
            nc.scalar.copy(aT16[0:10, :], aT_ps[0:10, :])
            wy16 = small_p.tile([128, 3072], BF16, tag="wy16")
            for s in range(6):
                wy_ps = ps_misc.tile([128, 512], FP32, tag="psmisc")
                nc.tensor.matmul(wy_ps, aT16[0:10, :], wall[0:10, s * 512:(s + 1) * 512],
                                 start=True, stop=True)
                nc.scalar.copy(wy16[:, s * 512:(s + 1) * 512], wy_ps)

            pt16 = g_p.tile([128, 2, 36, 128], BF16, tag="pt16")

            for half in range(2):
                c0 = half * 64
                # ---- mono tile [128n, 64c, 224m]
                mono = mono_p.tile([128, 64, MPAD], BF16, tag="mono")
                mv = mono[:, :, :]
                # zero-pad rows 165:168 (transposed into usymB zero rows)
                nc.gpsimd.memset(mono[:, :, 165:168], 0.0)
                nc.gpsimd.memset(mono[:, :, 222:256], 0.0)
                # x cast: mono[., c, 213:222] = x32[., (c0+c)*9 + w]
                nc.vector.tensor_copy(
                    mono[:, :, D1OFF:D1OFF + 9],
                    x32.rearrange("p (c w) -> p c w", w=9)[:, c0:c0 + 64, :])
                xs = mono[:, :, D1OFF:D1OFF + 9]      # [p, 64, 9] view
                # deg2: for v: mono[., c, D2OFF+tri(v)+u] = x_u * x_v (u<=v)
                for v in range(9):
                    out = mono[:, :, D2OFF + tri(v):D2OFF + tri(v) + v + 1]
                    in0 = xs[:, :, 0:v + 1]
                    in1 = xs[:, :, v:v + 1].broadcast_to((128, 64, v + 1))
                    nc.gpsimd.tensor_mul(out, in0, in1)
                # deg3: for v: mono[., c, B3+u*(9-v)+(w-v)] = xx_{u,v} * x_w (u<=v<=w)
                for v in range(9):
                    nu, nw = v + 1, 9 - v
                    out = mono[:, :, _B3[v]:_B3[v] + nu * nw].rearrange(
                        "p c (u w) -> p c u w", w=nw)
                    in0 = mono[:, :, D2OFF + tri(v):D2OFF + tri(v) + nu].unsqueeze(
                        3).broadcast_to((128, 64, nu, nw))
                    in1 = xs[:, :, v:9].unsqueeze(2).broadcast_to((128, 64, nu, nw))
                    nc.vector.tensor_mul(out, in0, in1)

                # ---- transpose to m-layout: monoT0 [128m, 64*128], monoT1 [94, ...]
                monoT0 = mt_p.tile([128, 64 * 128], BF16, tag="mT0")
                monoT1 = mt_p.tile([128, 64 * 128], BF16, tag="mT1")
                for ci in range(64):
                    nc.sync.dma_start_transpose(
                        monoT0[:, ci * 128:(ci + 1) * 128], mono[:, ci, 0:128])
                    nc.sync.dma_start_transpose(
                        monoT1[:, ci * 128:(ci + 1) * 128], mono[:, ci, 128:256])

                # ---- G matmul: 16 col-tiles of 512; pack 2 tiles per PSUM rows 0/64
                # g16 [128, 4096]: cols tpair*512+k; rows 0:64 = even tile, 64:128 odd
                g16 = g_p.tile([128, 4096], BF16, tag="g16")
                for tpair in range(8):
                    g_ps = ps_gps.tile([128, 512], FP32, tag="gps")
                    for sub in range(2):
                        t = tpair * 2 + sub
                        sl = slice(t * 512, (t + 1) * 512)
                        nc.tensor.matmul(g_ps[sub * 64:sub * 64 + 64, :],
                                         usymA[:, :], monoT0[:, sl],
                                         start=True, stop=False,
                                         skip_group_check=True)
                        nc.tensor.matmul(g_ps[sub * 64:sub * 64 + 64, :],
                                         usymB[0:94, :], monoT1[0:94, sl],
                                         start=False, stop=True,
                                         skip_group_check=True)
                    nc.scalar.copy(g16[:, tpair * 512:(tpair + 1) * 512], g_ps)

                # ---- transpose G back: gn16 [128n, 64c, 64j]
                gn16 = g_p.tile([128, 64, 64], BF16, tag="gn16")
                for ci in range(64):
                    t = ci // 4
                    rb = 64 * (t % 2)
                    cb = (t // 2) * 512 + (ci % 4) * 128
                    nc.sync.dma_start_transpose(
                        gn16[:, ci, :], g16[rb:rb + 64, cb:cb + 128])

                # ---- wyG: p16 [128n, 72j', 64c]  (j' = BL8[l] + d*8 + q8)
                p16 = small_p.tile([128, 72, 64], BF16, tag="p16")
                gfull = gn16[:, :, :]
                for li in range(3):
                    dl = DIMS[li]
                    out = p16[:, BL8[li]:BL8[li] + dl * 8, :].rearrange(
                        "p (d q) c -> p d q c", q=8)
                    in0 = _raw(gfull, BL[li],
                               [list(gfull.ap[0]), [7, dl], [1, 8], [64, 64]])
                    in1 = wy16.rearrange("p (l q c) -> p l q c", l=3, q=8)[
                        :, li, :, c0:c0 + 64].unsqueeze(1).broadcast_to(
                        (128, dl, 8, 64))
                    nc.vector.tensor_mul(out, in0, in1)

                # ---- PT: transpose q-pairs; pt_ps [128=(c64|c64), 128n] bf16
                jlist = []
                for li in range(3):
                    for d in range(DIMS[li]):
                        for qp in range(4):
                            jlist.append(BL8[li] + d * 8 + qp * 2)
                for batch in range(2):
                    pt_ps = ps_pt.tile([128, 18, 128], BF16, tag="ptps")
                    for kk in range(18):
                        j0 = jlist[batch * 18 + kk]
                        nc.tensor.transpose(
                            pt_ps[:, kk, :],
                            p16[:, j0:j0 + 2, :].rearrange("p a c -> p (a c)"),
                            id16, )
                    nc.scalar.copy(pt16[:, half, batch * 18:(batch + 1) * 18, :], pt_ps)

            # ---- z matmuls: per (l,d): 8 accumulating MMs over (half, qpair)
            z32 = io1.tile([128, 1152], FP32, tag="z32")
            k = 0
            for li in range(3):
                for d in range(DIMS[li]):
                    z_full = ps_misc.tile([128, 512], FP32, tag="psmisc")
                    z_ps = z_full[:, 0:128]
                    step = 0
                    for half in range(2):
                        for qp in range(4):
                            nc.tensor.matmul(
                                z_ps, lw2[:, (li * 2 + half) * 128:(li * 2 + half + 1) * 128],
                                pt16[:, half, k * 4 + qp, :],
                                start=(step == 0), stop=(step == 7),
                                skip_group_check=True)
                            step += 1
                    nc.scalar.copy(z32[:, k * 128:(k + 1) * 128], z_ps)
                    k += 1

            # ---- zT + assemble + sc + store
            out32 = io1.tile([128, 1152], FP32, tag="out32")
            kbase = [0, 128, 512]
            k = 0
            for li in range(3):
                dl = DIMS[li]
                for d in range(DIMS[li]):
                    zt_full = ps_misc.tile([128, 512], FP32, tag="psmisc")
                    zt_ps = zt_full[:, 0:128]
                    nc.tensor.transpose(zt_ps, z32[:, k * 128:(k + 1) * 128], id32)
                    o = _raw(out32[:, :], kbase[li] + d,
                             [list(out32[:, :].ap[0]), [dl, 128]])
                    s = _raw(sc32[:, :], kbase[li] + d,
                             [list(sc32[:, :].ap[0]), [dl, 128]])
                    nc.vector.tensor_add(o, zt_ps, s)
                    k += 1
            nc.sync.dma_start(out=yout[n0:n0 + 128, :], in_=out32)

    nc.compile()
    return nc


# --------------------------- public entry ---------------------------------

_PROG = None


def _get_prog():
    global _PROG
    if _PROG is None:
        _PROG = build_program()
    return _PROG


def host_constants(inputs):
    usym = build_usym(inputs)
    uA, uB = build_usym_tiles(usym)
    wall = build_wall(inputs)
    lw2 = build_lw2(inputs)
    ident = np.eye(128, dtype=np.float32)
    import ml_dtypes
    return {
        "usymA": uA.astype(ml_dtypes.bfloat16),
        "usymB": uB.astype(ml_dtypes.bfloat16),
        "wall": wall.astype(ml_dtypes.bfloat16),
        "lw2": lw2.astype(ml_dtypes.bfloat16),
        "id32": ident,
        "id16": ident.astype(ml_dtypes.bfloat16),
    }


def make_in_maps(inputs):
    consts = host_constants(inputs)
    nf = np.asarray(inputs["node_feats"], dtype=np.float32).reshape(N, 1152)
    sc = np.asarray(inputs["sc"], dtype=np.float32)
    at = np.asarray(inputs["node_attrs"], dtype=np.float32)
    at16 = np.zeros((N, 16), dtype=np.float32)
    at16[:, 0:10] = at
    in_maps = []
    for c in range(NCORES):
        sl = slice(c * NSH, (c + 1) * NSH)
        m = {"xin": np.ascontiguousarray(nf[sl]),
             "scin": np.ascontiguousarray(sc[sl]),
             "ain": np.ascontiguousarray(at16[sl])}
        m.update(consts)
        in_maps.append(m)
    return in_maps


def kernel(**inputs):
    nc = _get_prog()
    in_maps = make_in_maps(inputs)
    res = run_bass_kernel_spmd(nc, in_maps, list(range(NCORES)))
    out = np.concatenate([res.results[i]["yout"] for i in range(NCORES)], axis=0)
    return out.astype(np.float32)


# ----------------------- timing helpers (test.py only) ---------------------

def _build_runner(nc, in_maps):
    """Jitted 8-core sharded executor for an arbitrary bass program; returns
    (fn, device_args). No donation so the same device buffers can be reused."""
    import jax
    from jax.sharding import Mesh, PartitionSpec, NamedSharding
    from jax.experimental.shard_map import shard_map
    import concourse.mybir as mb
    from concourse import bass2jax

    bass2jax.install_neuronx_cc_hook()
    partition_name = nc.partition_id_tensor.name if nc.partition_id_tensor else None
    in_names, out_names, out_avals, zero_outs = [], [], [], []
    for alloc in nc.m.functions[0].allocations:
        if not isinstance(alloc, mb.MemoryLocationSet):
            continue
        name = alloc.memorylocations[0].name
        if alloc.kind == "ExternalInput":
            if name != partition_name:
                in_names.append(name)
        elif alloc.kind == "ExternalOutput":
            dt = mb.dt.np(alloc.dtype)
            out_avals.append(jax.core.ShapedArray(tuple(alloc.tensor_shape), dt))
            out_names.append(name)
            zero_outs.append(np.zeros(tuple(alloc.tensor_shape), dt))

    all_names = list(in_names) + list(out_names)
    if partition_name is not None:
        all_names.append(partition_name)

    def _body(*args):
        operands = list(args)
        if partition_name is not None:
            operands.append(bass2jax.partition_id_tensor())
        outs = bass2jax._bass_exec_p.bind(
            *operands,
            out_avals=tuple(out_avals),
            in_names=tuple(all_names),
            out_names=tuple(out_names),
            lowering_input_output_aliases=(),
            sim_require_finite=True,
            sim_require_nnan=True,
            nc=nc,
        )
        return tuple(outs)

    devices = jax.devices()[:NCORES]
    mesh = Mesh(np.asarray(devices), ("core",))
    nin = len(in_names) + len(zero_outs)
    fn = jax.jit(shard_map(_body, mesh=mesh,
                           in_specs=(PartitionSpec("core"),) * nin,
                           out_specs=(PartitionSpec("core"),) * len(out_names),
                           check_rep=False))
    sh = NamedSharding(mesh, PartitionSpec("core"))
    concat = [np.concatenate([m[n] for m in in_maps], axis=0) for n in in_names]
    concat += [np.concatenate([z] * NCORES, axis=0) for z in zero_outs]
    dargs = [jax.device_put(a, sh) for a in concat]
    return fn, dargs


def _build_trivial():
    """Minimal bass program for dispatch-overhead baseline."""
    nc = bacc.Bacc("TRN2", target_bir_lowering=False, debug=False)
    ti = nc.dram_tensor("tin", [128, 16], FP32, kind="ExternalInput").ap()
    to = nc.dram_tensor("tout", [128, 16], FP32, kind="ExternalOutput").ap()
    from contextlib import ExitStack
    with _TC(nc) as tc, ExitStack() as ctx:
        p = ctx.enter_context(tc.tile_pool(name="p", bufs=1))
        t = p.tile([128, 16], FP32)
        nc.sync.dma_start(out=t, in_=ti)
        nc.sync.dma_start(out=to, in_=t)
    nc.compile()
    return nc


def _time_fn(fn, dargs, iters):
    import time
    import jax
    o = fn(*dargs)
    jax.block_until_ready(o)
    best = float("inf")
    for _ in range(3):
        t0 = time.perf_counter()
        outs = [fn(*dargs) for _ in range(iters)]
        jax.block_until_ready(outs[-1])
        t1 = time.perf_counter()
        best = min(best, (t1 - t0) / iters)
    return best


def measure_hw_time(inputs, iters=32):
    nc = _get_prog()
    in_maps = make_in_maps(inputs)
    fn, dargs = _build_runner(nc, in_maps)
    t_full = _time_fn(fn, dargs, iters)

    tnc = _build_trivial()
    tmaps = [{"tin": np.zeros((128, 16), np.float32)} for _ in range(NCORES)]
    tfn, tdargs = _build_runner(tnc, tmaps)
    t_base = _time_fn(tfn, tdargs, iters)

    print(f"  per-call wall: full={t_full * 1e6:.1f}us base={t_base * 1e6:.1f}us")
    return max(t_full - t_base, 0.0) * 1e9


if __name__ == "__main__":
    nc = build_program()
    print("program built ok; instructions:",
          sum(len(b.instructions) for f in nc.m.functions for b in f.blocks))



# revision 10
# speedup vs baseline: 6.6074x; 6.6074x over previous
"""Trainium2 Bass kernel for nn_EquivariantProductBasisBlock.

Math: per (node n, channel c), out[d] is a degree-3 polynomial in the
9-vector x[n,c,:] with coefficients depending on (element_type(n), c),
followed by a per-l channel-mixing linear layer and the sc skip-add.

Device mapping (per core, 256 nodes, data-parallel over 8 cores):
  1. monomials (219 per (n,c)) on DVE/GPSIMD in node-partition layout
  2. DMA-xbar transpose to monomial-partition layout
  3. PE matmul with the symmetrized coefficient matrix Usym [219 x 63]
  4. DMA-xbar transpose back; multiply by per-(n,c) element weights (wy)
  5. PE transpose + lin_w matmul contracting channels; add sc; DMA out.
"""
import numpy as np
from itertools import permutations

import bass_rust
import concourse.bass as bass
import concourse.bacc as bacc
import concourse.tile as tile
import concourse.mybir as mybir
from concourse.bass_utils import run_bass_kernel_spmd
from concourse.vector_clock import ScopedClock

# ---------------- problem constants (hardcoded per contest rules) ----------
N, C, E = 2048, 128, 10
NCORES = 8
NSH = N // NCORES            # 256 nodes per core
DIMS = [1, 3, 5]
P1, P2, P3 = 1, 2, 4
BL = [0, 7, 28]              # 7-packed col base per l  (63 cols)
BL8 = [0, 8, 32]             # 8-packed col base per l  (72 cols)
FP32 = mybir.dt.float32
BF16 = mybir.dt.bfloat16

# monomial layout: monoA [128n, 64c, 128m] = deg3 rows 0:128;
# monoB [128n, 64c, 128m] = [0:37 deg3 tail | 37:82 deg2 | 82:91 deg1(x) | 91:128 pad]
D2OFF = 37   # inside monoB
D1OFF = 82   # inside monoB
# global usym row layout (219): [deg3 0:165 | deg2 165:210 | deg1 210:219]


def tri(v):
    return v * (v + 1) // 2


_B3 = [0]
for v in range(9):
    _B3.append(_B3[-1] + (v + 1) * (9 - v))
assert _B3[9] == 165


def m3_idx(u, v, w):
    return _B3[v] + u * (9 - v) + (w - v)


# ------------------------- host precompute --------------------------------

def build_usym(inputs):
    """usym [219, 64] float64; col j = BL[l] + d*7 + q (q: 0-3 nu3, 4-5 nu2, 6 nu1)."""
    usym = np.zeros((219, 64), dtype=np.float64)
    for li in range(3):
        U1 = np.asarray(inputs[f"U1_{li}"], dtype=np.float64)
        U2 = np.asarray(inputs[f"U2_{li}"], dtype=np.float64)
        U3 = np.asarray(inputs[f"U3_{li}"], dtype=np.float64)
        for d in range(DIMS[li]):
            base = BL[li] + d * 7
            for p in range(P3):
                T = U3[d, :, :, :, p]
                for u in range(9):
                    for v in range(u, 9):
                        for w in range(v, 9):
                            s = sum(T[pm] for pm in set(permutations((u, v, w))))
                            usym[m3_idx(u, v, w), base + p] = s
            for p in range(P2):
                T = U2[d, :, :, p]
                for u in range(9):
                    for v in range(u, 9):
                        s = sum(T[pm] for pm in set(permutations((u, v))))
                        usym[165 + tri(v) + u, base + 4 + p] = s
            for u in range(9):
                usym[210 + u, base + 6] = U1[d, u, 0]
    return usym


def build_usym_tiles(usym):
    """usymA [128,64] rows=m 0:128; usymB [128,64] rows 0:91 laid out to match
    monoB: [0:37 deg3 m128:165 | 37:82 deg2 | 82:91 deg1]."""
    a = np.zeros((128, 64), dtype=np.float32)
    b = np.zeros((128, 64), dtype=np.float32)
    a[:, :] = usym[0:128]
    b[0:37] = usym[128:165]
    b[37:82] = usym[165:210]
    b[82:91] = usym[210:219]
    return a, b


def build_wall(inputs):
    """wall [16, 3072] f32: rows 0:10 = element e; col = l*1024 + q*128 + c, q8 (q=7 zero)."""
    wall = np.zeros((16, 3072), dtype=np.float32)
    for li in range(3):
        W1 = np.asarray(inputs["W1"][li])  # [E,P1,C]
        W2 = np.asarray(inputs["W2"][li])
        W3 = np.asarray(inputs["W3"][li])
        for q in range(7):
            if q < 4:
                w = W3[:, q, :]
            elif q < 6:
                w = W2[:, q - 4, :]
            else:
                w = W1[:, 0, :]
            wall[0:E, li * 1024 + q * 128:li * 1024 + q * 128 + C] = w
    return wall


def build_lw2(inputs):
    """lw2 [128, 768] f32: six stationaries s=(l,half): col s*128+f;
    rows r: c = half*64 + (r % 64); value lin_w[l][c,f]/sqrt(C)."""
    lw2 = np.zeros((128, 768), dtype=np.float32)
    isq = 1.0 / np.sqrt(np.float32(C))
    lw = np.asarray(inputs["lin_w"])  # [3, C, C]
    for li in range(3):
        for half in range(2):
            s = li * 2 + half
            blk = lw[li, half * 64:(half + 1) * 64, :] * isq  # [64, 128]
            lw2[0:64, s * 128:(s + 1) * 128] = blk
            lw2[64:128, s * 128:(s + 1) * 128] = blk
    return lw2


# --------------------------- device program --------------------------------

class _TC(tile.TileContext):
    """TileContext with the final sync-engine drain split into 1-wait drains
    (this walrus build rejects >1 sem wait on a sync CTRL instruction)."""

    def _drain_and_barrier(self, tick_clock, wait_clock):
        drain_inst = self.nc.sync.drain()
        wait_clock.add_sem_waits(
            drain_inst.ins, ScopedClock({None: tick_clock.global_clock})
        )
        si = drain_inst.ins.sync_info
        waits = list(si.on_wait or []) if si else []
        if len(waits) > 1:
            si.on_wait = waits[:1]
            for w in waits[1:]:
                extra = self.nc.sync.drain()
                extra.ins.sync_info = bass_rust.SyncInfo(on_wait=[w], on_update=[])
        self.nc.all_engine_barrier()
        assert self.sems is not None
        popped = self.nc._tile_sem_poison_stack.pop()
        assert popped is self._sem_poison
        self.nc.clear_and_free_semaphores(list(self.sems.allocated().values()))
        self.nc.all_engine_barrier()


def _raw(ap_like, extra_offset, dims):
    """Build a raw AP on the same tensor: dims = [[step,count],...] incl. partition dim."""
    base = ap_like[:, :] if not isinstance(ap_like, bass.AP) else ap_like
    return bass.AP(tensor=base.tensor, offset=base.offset + extra_offset, ap=dims)


def _xbar(eng, out_ap, in_ap):
    """Raw batched xbar transpose: out[a,b,c] = in[c,(b,a)] with 3D APs.

    Same emit as eng.dma_start_transpose minus its 2D-input restriction;
    lower_ap_dma(opt=False) preserves the dim structure verbatim.
    """
    out_l = eng.lower_ap_dma(out_ap, for_isa=True)
    in_l = eng.lower_ap_dma(in_ap, for_isa=True)
    return eng.add_instruction(
        mybir.InstDmaTransposeAnt(
            name=eng.bass.get_next_instruction_name(),
            ins=in_l, outs=out_l,
            tile_src_rows=eng.bass.XBAR_TILE_SRC_ROWS,
            tile_src_cols=eng.bass.XBAR_TILE_SRC_COLS,
        ))


def build_program():
    nc = bacc.Bacc("TRN2", target_bir_lowering=False, debug=False)

    xin = nc.dram_tensor("xin", [NSH, 1152], FP32, kind="ExternalInput").ap()
    scin = nc.dram_tensor("scin", [NSH, 1152], FP32, kind="ExternalInput").ap()
    ain = nc.dram_tensor("ain", [NSH, 16], FP32, kind="ExternalInput").ap()
    usymA_d = nc.dram_tensor("usymA", [128, 64], BF16, kind="ExternalInput").ap()
    usymB_d = nc.dram_tensor("usymB", [128, 64], BF16, kind="ExternalInput").ap()
    wall_d = nc.dram_tensor("wall", [16, 3072], BF16, kind="ExternalInput").ap()
    lw2_d = nc.dram_tensor("lw2", [128, 768], BF16, kind="ExternalInput").ap()
    id32_d = nc.dram_tensor("id32", [128, 128], FP32, kind="ExternalInput").ap()
    id16_d = nc.dram_tensor("id16", [128, 128], BF16, kind="ExternalInput").ap()
    yout = nc.dram_tensor("yout", [NSH, 1152], FP32, kind="ExternalOutput").ap()

    from contextlib import ExitStack
    with _TC(nc) as tc, ExitStack() as ctx:
        consts = ctx.enter_context(tc.tile_pool(name="consts", bufs=1))
        usymA = consts.tile([128, 64], BF16)
        usymB = consts.tile([128, 64], BF16)
        wall = consts.tile([16, 3072], BF16)
        lw2 = consts.tile([128, 768], BF16)
        id32 = consts.tile([128, 128], FP32)
        id16 = consts.tile([128, 128], BF16)
        for t, d in [(usymA, usymA_d), (usymB, usymB_d), (wall, wall_d),
                     (lw2, lw2_d), (id32, id32_d), (id16, id16_d)]:
            nc.sync.dma_start(out=t, in_=d)

        io = ctx.enter_context(tc.tile_pool(name="io", bufs=2))
        io1 = ctx.enter_context(tc.tile_pool(name="io1", bufs=1))
        mono_p = ctx.enter_context(tc.tile_pool(name="mono", bufs=2))
        monoB_p = ctx.enter_context(tc.tile_pool(name="monoB", bufs=1))
        # persistent double-buffer for monoB so its zero pad (cols 91:128)
        # survives across chunk-half iterations after a single memset
        monoBs = [monoB_p.tile([128, 64, 128], BF16, tag=f"monoB{i}",
                               name=f"monoBt{i}")
                  for i in range(2)]
        for mb in monoBs:
            nc.gpsimd.memset(mb[:, :, 91:128], 0.0)
        mt_p = ctx.enter_context(tc.tile_pool(name="mt", bufs=1))
        g_p = ctx.enter_context(tc.tile_pool(name="g", bufs=1))
        small_p = ctx.enter_context(tc.tile_pool(name="small", bufs=1))
        ps_gps = ctx.enter_context(tc.tile_pool(name="psg", bufs=2, space="PSUM"))
        ps_pt = ctx.enter_context(tc.tile_pool(name="psp", bufs=1, space="PSUM"))
        ps_misc = ctx.enter_context(tc.tile_pool(name="psm", bufs=2, space="PSUM"))

        for chunk in range(2):
            n0 = chunk * 128
            # ---- loads
            x32 = io.tile([128, 1152], FP32, tag="x32")
            nc.sync.dma_start(out=x32, in_=xin[n0:n0 + 128, :])
            sc32 = io1.tile([128, 1152], FP32, tag="sc32")
            nc.sync.dma_start(out=sc32, in_=scin[n0:n0 + 128, :])
            a32 = io.tile([128, 16], FP32, tag="a32")
            nc.sync.dma_start(out=a32[:, 0:10], in_=ain[n0:n0 + 128, 0:10])

            # ---- wy = one_hot @ wall   -> wy16 [128n, 3072] (l,q8,c)
            aT_ps = ps_misc.tile([16, 128], FP32, tag="psmisc")
            nc.tensor.transpose(aT_ps[0:10, :], a32[:, 0:10], id32)
            aT16 = small_p.tile([16, 128], BF16, tag="aT16")
            nc.scalar.copy(aT16[0:10, :], aT_ps[0:10, :])
            wy16 = small_p.tile([128, 3072], BF16, tag="wy16")
            for s in range(6):
                wy_ps = ps_misc.tile([128, 512], FP32, tag="psmisc")
                nc.tensor.matmul(wy_ps, aT16[0:10, :], wall[0:10, s * 512:(s + 1) * 512],
                                 start=True, stop=True)
                nc.scalar.copy(wy16[:, s * 512:(s + 1) * 512], wy_ps)

            pt16 = g_p.tile([128, 2, 36, 128], BF16, tag="pt16")

            for half in range(2):
                c0 = half * 64
                # ---- mono tiles [128n, 64c, 128m] (contiguous per tile)
                monoA = mono_p.tile([128, 64, 128], BF16, tag="monoA")
                monoB = monoBs[(chunk * 2 + half) % 2]
                # x cast: monoB[., c, 82:91] = x32[., (c0+c)*9 + w]
                nc.vector.tensor_copy(
                    monoB[:, :, D1OFF:D1OFF + 9],
                    x32.rearrange("p (c w) -> p c w", w=9)[:, c0:c0 + 64, :])
                xs = monoB[:, :, D1OFF:D1OFF + 9]      # [p, 64, 9] view
                # deg2: monoB[., c, D2OFF+tri(v)+u] = x_u * x_v (u<=v)
                for v in range(9):
                    out = monoB[:, :, D2OFF + tri(v):D2OFF + tri(v) + v + 1]
                    in0 = xs[:, :, 0:v + 1]
                    in1 = xs[:, :, v:v + 1].broadcast_to((128, 64, v + 1))
                    nc.gpsimd.tensor_mul(out, in0, in1)
                # deg3: m3 = B3[v]+u*(9-v)+(w-v) = xx_{u,v} * x_w (u<=v<=w);
                # rows 0:128 -> monoA, 128:165 -> monoB rows 0:37 (v=6 splits)
                for v in range(9):
                    nu, nw = v + 1, 9 - v
                    d2 = monoB[:, :, D2OFF + tri(v):D2OFF + tri(v) + nu]
                    if v <= 5:
                        out = monoA[:, :, _B3[v]:_B3[v] + nu * nw].rearrange(
                            "p c (u w) -> p c u w", w=nw)
                        in0 = d2.unsqueeze(3).broadcast_to((128, 64, nu, nw))
                        in1 = xs[:, :, v:9].unsqueeze(2).broadcast_to(
                            (128, 64, nu, nw))
                        nc.vector.tensor_mul(out, in0, in1)
                    elif v == 6:
                        # m = 119 + u*3 + (w-6): u 0:3 -> monoA 119:128
                        out = monoA[:, :, 119:128].rearrange(
                            "p c (u w) -> p c u w", w=3)
                        in0 = d2[:, :, 0:3].unsqueeze(3).broadcast_to(
                            (128, 64, 3, 3))
                        in1 = xs[:, :, 6:9].unsqueeze(2).broadcast_to(
                            (128, 64, 3, 3))
                        nc.vector.tensor_mul(out, in0, in1)
                        # u 3:7 -> monoB 0:12
                        out = monoB[:, :, 0:12].rearrange(
                            "p c (u w) -> p c u w", w=3)
                        in0 = d2[:, :, 3:7].unsqueeze(3).broadcast_to(
                            (128, 64, 4, 3))
                        in1 = xs[:, :, 6:9].unsqueeze(2).broadcast_to(
                            (128, 64, 4, 3))
                        nc.vector.tensor_mul(out, in0, in1)
                    else:
                        # v=7: m 140:156 -> monoB 12:28; v=8: 156:165 -> 28:37
                        ob = _B3[v] - 128
                        out = monoB[:, :, ob:ob + nu * nw].rearrange(
                            "p c (u w) -> p c u w", w=nw)
                        in0 = d2.unsqueeze(3).broadcast_to((128, 64, nu, nw))
                        in1 = xs[:, :, v:9].unsqueeze(2).broadcast_to(
                            (128, 64, nu, nw))
                        nc.vector.tensor_mul(out, in0, in1)

                # ---- transpose to m-layout via batched xbar:
                # monoT[m, c, n] = mono[n, c, m]
                monoT0 = mt_p.tile([128, 64 * 128], BF16, tag="mT0")
                monoT1 = mt_p.tile([128, 64 * 128], BF16, tag="mT1")
                for mt, mo in ((monoT0, monoA), (monoT1, monoB)):
                    mv = mo[:, :, :]
                    _xbar(nc.sync,
                          _raw(mt[:, :], 0,
                               [list(mt[:, :].ap[0]), [128, 64], [1, 128]]),
                          _raw(mv, 0, [list(mv.ap[0]), [128, 64], [1, 128]]))

                # ---- G matmul: 16 col-tiles of 512; pack 2 per PSUM rows 0/64.
                # c-group g = sub*8 + tp so that PSUM rows 0:64 hold c 0:32 and
                # rows 64:128 hold c 32:64 across the 8 drained col-blocks.
                g16 = g_p.tile([128, 4096], BF16, tag="g16")
                for tp in range(8):
                    g_ps = ps_gps.tile([128, 512], FP32, tag="gps")
                    for sub in range(2):
                        g = sub * 8 + tp
                        sl = slice(g * 512, (g + 1) * 512)
                        nc.tensor.matmul(g_ps[sub * 64:sub * 64 + 64, :],
                                         usymA[:, :], monoT0[:, sl],
                                         start=True, stop=False,
                                         skip_group_check=True)
                        nc.tensor.matmul(g_ps[sub * 64:sub * 64 + 64, :],
                                         usymB[0:91, :], monoT1[0:91, sl],
                                         start=False, stop=True,
                                         skip_group_check=True)
                    nc.scalar.copy(g16[:, tp * 512:(tp + 1) * 512], g_ps)

                # ---- transpose G back: gn16 [128n, 64c, 64j]
                # batched xbar per sub-half: in [64j, (c32, n128)] -> out [n, c, j]
                gn16 = g_p.tile([128, 64, 64], BF16, tag="gn16")
                for sub in range(2):
                    gbase = g16[sub * 64:sub * 64 + 64, :]
                    _xbar(nc.sync,
                          _raw(gn16[:, :, :], sub * 32 * 64,
                               [list(gn16[:, :, :].ap[0]), [64, 32], [1, 64]]),
                          _raw(gbase, 0, [list(gbase.ap[0]), [128, 32], [1, 128]]))

                # ---- wyG: p16 [128n, 72j', 64c]  (j' = BL8[l] + d*8 + q8)
                p16 = small_p.tile([128, 72, 64], BF16, tag="p16")
                gfull = gn16[:, :, :]
                for li in range(3):
                    dl = DIMS[li]
                    out = p16[:, BL8[li]:BL8[li] + dl * 8, :].rearrange(
                        "p (d q) c -> p d q c", q=8)
                    in0 = _raw(gfull, BL[li],
                               [list(gfull.ap[0]), [7, dl], [1, 8], [64, 64]])
                    in1 = wy16.rearrange("p (l q c) -> p l q c", l=3, q=8)[
                        :, li, :, c0:c0 + 64].unsqueeze(1).broadcast_to(
                        (128, dl, 8, 64))
                    nc.vector.tensor_mul(out, in0, in1)

                # ---- PT: transpose q-pairs; pt_ps [128=(c64|c64), 128n] bf16
                jlist = []
                for li in range(3):
                    for d in range(DIMS[li]):
                        for qp in range(4):
                            jlist.append(BL8[li] + d * 8 + qp * 2)
                for batch in range(2):
                    pt_ps = ps_pt.tile([128, 18, 128], BF16, tag="ptps")
                    for kk in range(18):
                        j0 = jlist[batch * 18 + kk]
                        nc.tensor.transpose(
                            pt_ps[:, kk, :],
                            p16[:, j0:j0 + 2, :].rearrange("p a c -> p (a c)"),
                            id16, )
                    nc.scalar.copy(pt16[:, half, batch * 18:(batch + 1) * 18, :], pt_ps)

            # ---- z matmuls: per (l,d): 8 accumulating MMs over (half, qpair)
            z32 = io1.tile([128, 1152], FP32, tag="z32")
            k = 0
            for li in range(3):
                for d in range(DIMS[li]):
                    z_full = ps_misc.tile([128, 512], FP32, tag="psmisc")
                    z_ps = z_full[:, 0:128]
                    step = 0
                    for half in range(2):
                        for qp in range(4):
                            nc.tensor.matmul(
                                z_ps, lw2[:, (li * 2 + half) * 128:(li * 2 + half + 1) * 128],
                                pt16[:, half, k * 4 + qp, :],
                                start=(step == 0), stop=(step == 7),
                                skip_group_check=True)
                            step += 1
                    nc.scalar.copy(z32[:, k * 128:(k + 1) * 128], z_ps)
                    k += 1

            # ---- zT + assemble + sc + store
            out32 = io1.tile([128, 1152], FP32, tag="out32")
            kbase = [0, 128, 512]
            k = 0
            for li in range(3):
                dl = DIMS[li]
                for d in range(DIMS[li]):
                    zt_full = ps_misc.tile([128, 512], FP32, tag="psmisc")
                    zt_ps = zt_full[:, 0:128]
                    nc.tensor.transpose(zt_ps, z32[:, k * 128:(k + 1) * 128], id32)
                    o = _raw(out32[:, :], kbase[li] + d,
                             [list(out32[:, :].ap[0]), [dl, 128]])
                    s = _raw(sc32[:, :], kbase[li] + d,
                             [list(sc32[:, :].ap[0]), [dl, 128]])
                    nc.vector.tensor_add(o, zt_ps, s)
                    k += 1
            nc.sync.dma_start(out=yout[n0:n0 + 128, :], in_=out32)

    nc.compile()
    return nc


# --------------------------- public entry ---------------------------------

_PROG = None


def _get_prog():
    global _PROG
    if _PROG is None:
        _PROG = build_program()
    return _PROG


def host_constants(inputs):
    usym = build_usym(inputs)
    uA, uB = build_usym_tiles(usym)
    wall = build_wall(inputs)
    lw2 = build_lw2(inputs)
    ident = np.eye(128, dtype=np.float32)
    import ml_dtypes
    return {
        "usymA": uA.astype(ml_dtypes.bfloat16),
        "usymB": uB.astype(ml_dtypes.bfloat16),
        "wall": wall.astype(ml_dtypes.bfloat16),
        "lw2": lw2.astype(ml_dtypes.bfloat16),
        "id32": ident,
        "id16": ident.astype(ml_dtypes.bfloat16),
    }


def make_in_maps(inputs):
    consts = host_constants(inputs)
    nf = np.asarray(inputs["node_feats"], dtype=np.float32).reshape(N, 1152)
    sc = np.asarray(inputs["sc"], dtype=np.float32)
    at = np.asarray(inputs["node_attrs"], dtype=np.float32)
    at16 = np.zeros((N, 16), dtype=np.float32)
    at16[:, 0:10] = at
    in_maps = []
    for c in range(NCORES):
        sl = slice(c * NSH, (c + 1) * NSH)
        m = {"xin": np.ascontiguousarray(nf[sl]),
             "scin": np.ascontiguousarray(sc[sl]),
             "ain": np.ascontiguousarray(at16[sl])}
        m.update(consts)
        in_maps.append(m)
    return in_maps


def kernel(**inputs):
    nc = _get_prog()
    in_maps = make_in_maps(inputs)
    res = run_bass_kernel_spmd(nc, in_maps, list(range(NCORES)))
    out = np.concatenate([res.results[i]["yout"] for i in range(NCORES)], axis=0)
    return out.astype(np.float32)


# ----------------------- timing helpers (test.py only) ---------------------

def _build_runner(nc, in_maps):
    """Jitted 8-core sharded executor for an arbitrary bass program; returns
    (fn, device_args). No donation so the same device buffers can be reused."""
    import jax
    from jax.sharding import Mesh, PartitionSpec, NamedSharding
    from jax.experimental.shard_map import shard_map
    import concourse.mybir as mb
    from concourse import bass2jax

    bass2jax.install_neuronx_cc_hook()
    partition_name = nc.partition_id_tensor.name if nc.partition_id_tensor else None
    in_names, out_names, out_avals, zero_outs = [], [], [], []
    for alloc in nc.m.functions[0].allocations:
        if not isinstance(alloc, mb.MemoryLocationSet):
            continue
        name = alloc.memorylocations[0].name
        if alloc.kind == "ExternalInput":
            if name != partition_name:
                in_names.append(name)
        elif alloc.kind == "ExternalOutput":
            dt = mb.dt.np(alloc.dtype)
            out_avals.append(jax.core.ShapedArray(tuple(alloc.tensor_shape), dt))
            out_names.append(name)
            zero_outs.append(np.zeros(tuple(alloc.tensor_shape), dt))

    all_names = list(in_names) + list(out_names)
    if partition_name is not None:
        all_names.append(partition_name)

    def _body(*args):
        operands = list(args)
        if partition_name is not None:
            operands.append(bass2jax.partition_id_tensor())
        outs = bass2jax._bass_exec_p.bind(
            *operands,
            out_avals=tuple(out_avals),
            in_names=tuple(all_names),
            out_names=tuple(out_names),
            lowering_input_output_aliases=(),
            sim_require_finite=True,
            sim_require_nnan=True,
            nc=nc,
        )
        return tuple(outs)

    devices = jax.devices()[:NCORES]
    mesh = Mesh(np.asarray(devices), ("core",))
    nin = len(in_names) + len(zero_outs)
    fn = jax.jit(shard_map(_body, mesh=mesh,
                           in_specs=(PartitionSpec("core"),) * nin,
                           out_specs=(PartitionSpec("core"),) * len(out_names),
                           check_rep=False))
    sh = NamedSharding(mesh, PartitionSpec("core"))
    concat = [np.concatenate([m[n] for m in in_maps], axis=0) for n in in_names]
    concat += [np.concatenate([z] * NCORES, axis=0) for z in zero_outs]
    dargs = [jax.device_put(a, sh) for a in concat]
    return fn, dargs


def _build_trivial():
    """Minimal bass program for dispatch-overhead baseline."""
    nc = bacc.Bacc("TRN2", target_bir_lowering=False, debug=False)
    ti = nc.dram_tensor("tin", [128, 16], FP32, kind="ExternalInput").ap()
    to = nc.dram_tensor("tout", [128, 16], FP32, kind="ExternalOutput").ap()
    from contextlib import ExitStack
    with _TC(nc) as tc, ExitStack() as ctx:
        p = ctx.enter_context(tc.tile_pool(name="p", bufs=1))
        t = p.tile([128, 16], FP32)
        nc.sync.dma_start(out=t, in_=ti)
        nc.sync.dma_start(out=to, in_=t)
    nc.compile()
    return nc


def _time_fn(fn, dargs, iters):
    import time
    import jax
    o = fn(*dargs)
    jax.block_until_ready(o)
    best = float("inf")
    for _ in range(3):
        t0 = time.perf_counter()
        outs = [fn(*dargs) for _ in range(iters)]
        jax.block_until_ready(outs[-1])
        t1 = time.perf_counter()
        best = min(best, (t1 - t0) / iters)
    return best


def measure_hw_time(inputs, iters=32):
    nc = _get_prog()
    in_maps = make_in_maps(inputs)
    fn, dargs = _build_runner(nc, in_maps)
    t_full = _time_fn(fn, dargs, iters)

    tnc = _build_trivial()
    tmaps = [{"tin": np.zeros((128, 16), np.float32)} for _ in range(NCORES)]
    tfn, tdargs = _build_runner(tnc, tmaps)
    t_base = _time_fn(tfn, tdargs, iters)

    print(f"  per-call wall: full={t_full * 1e6:.1f}us base={t_base * 1e6:.1f}us")
    return max(t_full - t_base, 0.0) * 1e9


if __name__ == "__main__":
    nc = build_program()
    print("program built ok; instructions:",
          sum(len(b.instructions) for f in nc.m.functions for b in f.blocks))

